# revision 42
# baseline (speedup 1.0000x reference)
"""Trainium2 Bass kernel for nn_DLI_loss_full.

Key algebraic fact: logits[b,j,k] = hw[b,j] + xw[b,k] and the loss is
sum(lse - tgt) over valid groups, so the hw[b,j] term (the whole LSTM
path) cancels exactly:

    per_group[b,j] = log(sum_{k=j+1}^{len_b-1} exp(xw[b,k])) - xw[b,j+1]
    loss = sum(per_group) / sum_b(len_b - 1)

with xw = encoder_output @ w_fc[HID:].

Work packing (the big lever): the ragged mask is host-visible and the
sequences average ~len 193 of 384, so ~45% of encoder_output is fully
masked.  The host splits every sequence into G=32-timestep sub-chunks,
keeps only sub-chunks that intersect [0, len), balances whole batches
across the 8 cores by sub-chunk count, and packs each core's
sub-chunks into its 128 SBUF partition rows (zero-padded).  Per-core
stream and per-lane DVE work both drop by ~len/384 on average.  The
suffix-logsumexp seed for a sub-chunk is the sum of the totals of the
SAME batch's later sub-chunks: the host-built [128,128] matrix um
(um[i,j] = same batch and subchunk_i > subchunk_j) turns that into one
PE matmul, exactly as before at chunk granularity.

Per-core pipeline (all trace-driven):
  * stream DMAs cast f32->bf16 in the SDMA datapath (SWDGE path) so
    the multiply runs on DVE in bf16 2x mode and the 256->1 reduction
    is two bf16 2x tree-add halvings plus a 64-wide tensor_reduce.
  * w rides the scalar HWDGE queue ahead of the stream; the mask pack
    shares the stream's SWDGE queue right after piece 0 (a minority
    queue is starved to single-digit B/ns while the stream runs).
  * piece sizes are non-uniform: small first piece starts the DVE
    pipeline early, big middle pieces amortize the ~0.5us/piece DVE
    instruction overhead, tiny last pieces shrink the after-last-byte
    tail.  gpsimd gets NO elementwise work (a concurrent gpsimd
    tensor_tensor halves DVE 2x throughput, measured).
  * the mask is folded in additively ((mf-1)*30) before a single
    exp-with-accumulate: the accum IS the row total and masked exps
    are e^-30 so every suffix ln stays finite (zero-padded rows too).
  * exp and ln share one activation-table set (the act-table pass is
    steered to natural_log_exp_and_others), so no table load lands
    between the exp and the ln.
"""

from contextlib import ExitStack

import numpy as np

import concourse.bacc as bacc
import concourse.mybir as mybir
import concourse.tile as tile
from concourse import bass_utils

B, T, D, HID = 128, 384, 256, 256
NCORES = 8
P = 128                     # partition rows = packed sub-chunk slots
F32 = mybir.dt.float32
BF16 = mybir.dt.bfloat16
NEGM = 30.0                 # additive mask depth: exp(xw-30) ~ 1e-13

# piece splits (timesteps per piece) per sub-chunk size
_SZ = {
    28: (4, 8, 8, 4, 3, 1),
    32: (4, 8, 8, 8, 3, 1),
    40: (4, 9, 9, 9, 5, 3, 1),
    48: (4, 10, 10, 10, 10, 3, 1),
}

_cache = {}


def _joint_act_tables(arch, _orig=bacc.get_activation_tables):
    """Steer the act-table-load pass to the single set that holds BOTH
    exp and ln; per-function greedy choice otherwise puts a ~1.3us
    table load between the exp and the ln.  Names/order (and therefore
    act_func_set_ids) are preserved; other sets are emptied so they
    can never be chosen.  Falls back untouched if no joint set."""
    d = _orig(arch)
    exp = mybir.ActivationFunctionType.Exp
    ln = mybir.ActivationFunctionType.Ln
    joint = [n for n, fns in d.items() if exp in fns and ln in fns]
    if joint:
        keep = joint[0]
        for n in d:
            if n != keep:
                d[n] = set()
    return d


bacc.get_activation_tables = _joint_act_tables


def _build_nc(G, PX):
    SZ = _SZ[G]
    OFF = np.cumsum((0,) + SZ)[:len(SZ)]
    NP = len(SZ)
    MAXK = max(SZ)
    # pack layout, in bf16 columns
    PK_UM = 0
    PK_MF = PK_UM + P
    PK_WM = PK_MF + 2 * G
    PK_AM = PK_WM + 2 * G
    PK_N = PK_AM + 2 * G

    nc = bacc.Bacc(
        "TRN2", target_bir_lowering=False, debug=False, num_devices=NCORES
    )
    # the host pack already rewrites every byte of x, so it casts to
    # bf16 there: the DRAM stream halves (the SDMA was casting
    # f32->bf16 on the fly anyway, so SBUF contents are identical)
    x = nc.dram_tensor("x", [PX, G * D], BF16, kind="ExternalInput").ap()
    pw = nc.dram_tensor("pw", [P, D], BF16, kind="ExternalInput").ap()
    pk = nc.dram_tensor("pk", [P, PK_N], BF16, kind="ExternalInput").ap()
    out = nc.dram_tensor("out", [P, 2], F32, kind="ExternalOutput").ap()

    add = mybir.AluOpType.add
    mult = mybir.AluOpType.mult
    bypass = mybir.AluOpType.bypass
    AX = mybir.AxisListType.X
    ACT = mybir.ActivationFunctionType

    with tile.TileContext(nc) as tc, ExitStack() as ctx:
        sp = ctx.enter_context(tc.tile_pool(name="small", bufs=1))
        xp = ctx.enter_context(tc.tile_pool(name="xp", bufs=NP))
        hp = ctx.enter_context(tc.tile_pool(name="hp", bufs=3))
        pp = ctx.enter_context(tc.tile_pool(name="psum", bufs=1, space="PSUM"))

        pws = sp.tile([P, D], BF16)
        nc.scalar.dma_start(pws[:], pw)
        xts = []
        xt0 = xp.tile([PX, SZ[0] * D], BF16, tag="x")
        nc.gpsimd.dma_start(xt0[:], x[:, 0:SZ[0] * D])
        xts.append(xt0)
        pks = sp.tile([P, PK_N], BF16)
        nc.gpsimd.dma_start(pks[:], pk)
        for i in range(1, NP):
            xt = xp.tile([PX, SZ[i] * D], BF16, tag="x")
            nc.gpsimd.dma_start(
                xt[:], x[:, OFF[i] * D:(OFF[i] + SZ[i]) * D]
            )
            xts.append(xt)

        umv = pks[:, PK_UM:PK_UM + P]
        mfv = pks[:, PK_MF:PK_MF + 2 * G].bitcast(F32)
        wmv = pks[:, PK_WM:PK_WM + 2 * G].bitcast(F32)
        amv = pks[:, PK_AM:PK_AM + 2 * G].bitcast(F32)

        # activation-table warm: no data deps, runs in the DMA shadow
        warm0 = sp.tile([P, 1], F32)
        nc.vector.memset(warm0[:], 1.0)
        warmo = sp.tile([P, 2], F32)
        nc.scalar.activation(warmo[:, 0:1], warm0[:], ACT.Exp)

        # replicate w MAXK times on-chip (bf16 copies run at 4x)
        wrep = sp.tile([P, MAXK * D], BF16)
        nc.vector.tensor_copy(wrep[:, 0:D], pws[:])
        rep = 1
        while rep < MAXK:
            n = min(rep, MAXK - rep)
            nc.vector.tensor_copy(
                wrep[:, rep * D:(rep + n) * D], wrep[:, 0:n * D]
            )
            rep += n
        w3 = wrep[:].rearrange("p (l d) -> p l d", d=D)

        # xw[p, t] = sum_d x[p, t, d] * w[d].  Only the PX occupied
        # partition rows are streamed/multiplied; the zero-padded rest
        # of xw is memset once so the exp sees finite values there.
        xw = sp.tile([P, G], F32)
        res = sp.tile([P, 2], F32)
        if PX < P:
            nc.vector.memset(xw[:], 0.0)
        for i in range(NP):
            k = SZ[i]
            x3 = xts[i][:].rearrange("p (l d) -> p l d", d=D)
            nc.vector.tensor_tensor(x3, x3, w3[0:PX, 0:k, :], mult)
            if k <= 2:
                nc.vector.tensor_reduce(
                    xw[0:PX, OFF[i]:OFF[i] + k], x3, axis=AX, op=add
                )
            else:
                h1 = hp.tile([PX, MAXK * 128], BF16, tag="h1")
                h13 = h1[:, 0:k * 128].rearrange("p (l d) -> p l d", d=128)
                h2 = hp.tile([PX, MAXK * 64], BF16, tag="h2")
                h23 = h2[:, 0:k * 64].rearrange("p (l d) -> p l d", d=64)
                nc.vector.tensor_tensor(h13, x3[:, :, 0:128], x3[:, :, 128:256], add)
                nc.vector.tensor_tensor(h23, h13[:, :, 0:64], h13[:, :, 64:128], add)
                nc.vector.tensor_reduce(
                    xw[0:PX, OFF[i]:OFF[i] + k], h23, axis=AX, op=add
                )
            if i == 2:
                nc.vector.tensor_reduce(res[:, 1:2], mfv, axis=AX, op=add)

        # fold the mask in additively; padded rows carry x=0 -> xw=0,
        # amask=-30 -> exp ~1e-13, so everything stays finite
        nc.vector.tensor_tensor(xw[:], xw[:], amv, add)

        em = sp.tile([P, G], F32)
        tot = sp.tile([P, 1], F32)
        nc.scalar.activation(em[:], xw[:], ACT.Exp, accum_out=tot[:])

        # cross-sub-chunk exclusive suffix of totals via one bf16 matmul
        tot_bf = sp.tile([P, 1], BF16)
        nc.vector.tensor_copy(tot_bf[:], tot[:])
        aps = pp.tile([P, 1], F32, tag="mm")
        nc.tensor.matmul(aps[:], umv, tot_bf[:], start=True, stop=True)

        # within-sub-chunk suffix sums, seeded from PSUM
        ss = sp.tile([P, G], F32)
        nc.vector.tensor_tensor_scan(
            ss[:][:, ::-1], em[:][:, ::-1], em[:][:, ::-1],
            initial=aps[:], op0=add, op1=bypass,
        )
        lt = sp.tile([P, G], F32)
        nc.scalar.activation(lt[:], ss[:], ACT.Ln)

        diff = sp.tile([P, G], F32)
        nc.vector.tensor_sub(diff[:], lt[:], xw[:])
        nc.vector.scalar_tensor_tensor(
            out=diff[:], in0=diff[:], scalar=1.0, in1=wmv,
            op0=bypass, op1=mult, accum_out=res[:, 0:1],
        )
        nc.sync.dma_start(out, res[:], single_packet=True)

    nc.compile()
    return nc


def make_in_maps(enc, mask, w_fc):
    import ml_dtypes

    bf = ml_dtypes.bfloat16
    lens = mask.sum(axis=1).astype(np.int64)          # [B]

    # pick the smallest sub-chunk size whose balanced packing fits the
    # 128 slots per core; batches stay whole on one core
    for G in sorted(_SZ):
        nch = -(-lens // G)                            # ceil(len/G) per batch
        order = np.argsort(-nch)
        core_of = np.empty(B, np.int64)
        load = np.zeros(NCORES, np.int64)
        ok = True
        for b in order:
            c = int(np.argmin(load))
            if load[c] + nch[b] > P:
                ok = False
                break
            core_of[b] = c
            load[c] += nch[b]
        if ok:
            break
    assert ok, "no sub-chunk size fits the 128 slots per core"
    # stream only occupied partition rows (multiple of 16 for the DMA
    # engine swizzle)
    PX = min(P, int(-(-int(load.max()) // 16) * 16))

    w_bits = np.tile(w_fc[HID:].astype(bf).view(np.uint16)[None, :], (P, 1))
    PK_UM = 0
    PK_MF = PK_UM + P
    PK_WM = PK_MF + 2 * G
    PK_AM = PK_WM + 2 * G
    PK_N = PK_AM + 2 * G

    in_maps = []
    for c in range(NCORES):
        slots = [
            (b, sc)
            for b in range(B) if core_of[b] == c
            for sc in range(int(nch[b]))
        ]
        xbuf = np.zeros((PX, G, D), np.float32)
        mf = np.zeros((P, G), np.float32)
        wm = np.zeros((P, G), np.float32)
        sb = np.full(P, -1, np.int64)
        ssc = np.zeros(P, np.int64)
        for s, (b, sc) in enumerate(slots):
            t0 = sc * G
            t1 = min(t0 + G, T)
            xbuf[s, 0:t1 - t0] = enc[b, t0:t1]
            tt = np.arange(t0, t0 + G)
            mf[s] = (tt < lens[b]).astype(np.float32)
            wm[s] = ((tt >= 1) & (tt < lens[b])).astype(np.float32)
            sb[s] = b
            ssc[s] = sc
        am = (mf - 1.0) * NEGM
        um = (
            (sb[:, None] == sb[None, :]) & (sb[:, None] >= 0)
            & (ssc[:, None] > ssc[None, :])
        ).astype(np.float32)
        pack = np.empty((P, PK_N), np.uint16)
        pack[:, PK_UM:PK_UM + P] = um.astype(bf).view(np.uint16)
        pack[:, PK_MF:PK_MF + 2 * G] = mf.view(np.uint16)
        pack[:, PK_WM:PK_WM + 2 * G] = wm.view(np.uint16)
        pack[:, PK_AM:PK_AM + 2 * G] = am.view(np.uint16)
        in_maps.append({
            "x": np.ascontiguousarray(xbuf.reshape(PX, G * D).astype(bf)),
            "pw": w_bits.view(bf),
            "pk": pack.view(bf),
        })
    return G, PX, in_maps


def kernel(**inputs) -> np.ndarray:
    enc = np.ascontiguousarray(np.asarray(inputs["encoder_output"], np.float32))
    mask = np.ascontiguousarray(np.asarray(inputs["mask"], np.int32))
    w_fc = np.asarray(inputs["w_fc"], np.float32)

    G, PX, in_maps = make_in_maps(enc, mask, w_fc)
    if (G, PX) not in _cache:
        _cache[(G, PX)] = _build_nc(G, PX)
    nc = _cache[(G, PX)]

    res = bass_utils.run_bass_kernel_spmd(
        nc, in_maps, core_ids=list(range(NCORES))
    )
    o = np.stack([r["out"] for r in res.results]).astype(np.float64)
    num = o[:, :, 0].sum()
    den = o[:, :, 1].sum() - B
    return np.asarray(num / den, dtype=np.float32)


# revision 44
# speedup vs baseline: 1.0535x; 1.0535x over previous
"""Trainium2 Bass kernel for nn_DLI_loss_full.

Key algebraic fact: logits[b,j,k] = hw[b,j] + xw[b,k] and the loss is
sum(lse - tgt) over valid groups, so the hw[b,j] term (the whole LSTM
path) cancels exactly:

    per_group[b,j] = log(sum_{k=j+1}^{len_b-1} exp(xw[b,k])) - xw[b,j+1]
    loss = sum(per_group) / sum_b(len_b - 1)

with xw = encoder_output @ w_fc[HID:].

Work packing (the big lever): the ragged mask is host-visible and the
sequences average ~len 193 of 384, so ~45% of encoder_output is fully
masked.  The host splits every sequence into G=32-timestep sub-chunks,
keeps only sub-chunks that intersect [0, len), balances whole batches
across the 8 cores by sub-chunk count, and packs each core's
sub-chunks into its 128 SBUF partition rows (zero-padded).  Per-core
stream and per-lane DVE work both drop by ~len/384 on average.  The
suffix-logsumexp seed for a sub-chunk is the sum of the totals of the
SAME batch's later sub-chunks: the host-built [128,128] matrix um
(um[i,j] = same batch and subchunk_i > subchunk_j) turns that into one
PE matmul, exactly as before at chunk granularity.

Per-core pipeline (all trace-driven):
  * stream DMAs cast f32->bf16 in the SDMA datapath (SWDGE path) so
    the multiply runs on DVE in bf16 2x mode and the 256->1 reduction
    is two bf16 2x tree-add halvings plus a 64-wide tensor_reduce.
  * w rides the scalar HWDGE queue ahead of the stream; the mask pack
    shares the stream's SWDGE queue right after piece 0 (a minority
    queue is starved to single-digit B/ns while the stream runs).
  * piece sizes are non-uniform: small first piece starts the DVE
    pipeline early, big middle pieces amortize the ~0.5us/piece DVE
    instruction overhead, tiny last pieces shrink the after-last-byte
    tail.  gpsimd gets NO elementwise work (a concurrent gpsimd
    tensor_tensor halves DVE 2x throughput, measured).
  * the mask is folded in additively ((mf-1)*30) before a single
    exp-with-accumulate: the accum IS the row total and masked exps
    are e^-30 so every suffix ln stays finite (zero-padded rows too).
  * exp and ln share one activation-table set (the act-table pass is
    steered to natural_log_exp_and_others), so no table load lands
    between the exp and the ln.
"""

from contextlib import ExitStack

import numpy as np

import concourse.bacc as bacc
import concourse.mybir as mybir
import concourse.tile as tile
from concourse import bass_utils

B, T, D, HID = 128, 384, 256, 256
NCORES = 8
P = 128                     # partition rows = packed sub-chunk slots
F32 = mybir.dt.float32
BF16 = mybir.dt.bfloat16
NEGM = 30.0                 # additive mask depth: exp(xw-30) ~ 1e-13

# piece splits (timesteps per piece) per sub-chunk size
_SZ = {
    28: (4, 8, 8, 7, 1),
    32: (4, 8, 8, 8, 3, 1),
    40: (4, 9, 9, 9, 5, 3, 1),
    48: (4, 10, 10, 10, 10, 3, 1),
}

_cache = {}


def _joint_act_tables(arch, _orig=bacc.get_activation_tables):
    """Steer the act-table-load pass to the single set that holds BOTH
    exp and ln; per-function greedy choice otherwise puts a ~1.3us
    table load between the exp and the ln.  Names/order (and therefore
    act_func_set_ids) are preserved; other sets are emptied so they
    can never be chosen.  Falls back untouched if no joint set."""
    d = _orig(arch)
    exp = mybir.ActivationFunctionType.Exp
    ln = mybir.ActivationFunctionType.Ln
    joint = [n for n, fns in d.items() if exp in fns and ln in fns]
    if joint:
        keep = joint[0]
        for n in d:
            if n != keep:
                d[n] = set()
    return d


bacc.get_activation_tables = _joint_act_tables


def _build_nc(G, PX):
    SZ = _SZ[G]
    OFF = np.cumsum((0,) + SZ)[:len(SZ)]
    NP = len(SZ)
    MAXK = max(SZ)
    # pack layout, in bf16 columns
    PK_UM = 0
    PK_MF = PK_UM + P
    PK_WM = PK_MF + 2 * G
    PK_AM = PK_WM + 2 * G
    PK_N = PK_AM + 2 * G

    nc = bacc.Bacc(
        "TRN2", target_bir_lowering=False, debug=False, num_devices=NCORES
    )
    # the host pack already rewrites every byte of x, so it casts to
    # bf16 there: the DRAM stream halves (the SDMA was casting
    # f32->bf16 on the fly anyway, so SBUF contents are identical)
    x = nc.dram_tensor("x", [PX, G * D], BF16, kind="ExternalInput").ap()
    pw = nc.dram_tensor("pw", [P, D], BF16, kind="ExternalInput").ap()
    pk = nc.dram_tensor("pk", [P, PK_N], BF16, kind="ExternalInput").ap()
    out = nc.dram_tensor("out", [P, 2], F32, kind="ExternalOutput").ap()

    add = mybir.AluOpType.add
    mult = mybir.AluOpType.mult
    bypass = mybir.AluOpType.bypass
    AX = mybir.AxisListType.X
    ACT = mybir.ActivationFunctionType

    with tile.TileContext(nc) as tc, ExitStack() as ctx:
        sp = ctx.enter_context(tc.tile_pool(name="small", bufs=1))
        xp = ctx.enter_context(tc.tile_pool(name="xp", bufs=NP))
        hp = ctx.enter_context(tc.tile_pool(name="hp", bufs=3))
        pp = ctx.enter_context(tc.tile_pool(name="psum", bufs=1, space="PSUM"))

        # with the host pre-casting to bf16 there is no SDMA cast, so
        # the whole stream rides the scalar HWDGE queue: RTL descriptor
        # generation and ~0.6us first-byte latency vs the SWDGE Q7
        # software path's ~1.3us.  Strict FIFO: w, x piece 0, the mask
        # pack, then the remaining pieces.
        pws = sp.tile([P, D], BF16)
        nc.scalar.dma_start(pws[:], pw)
        xts = []
        xt0 = xp.tile([PX, SZ[0] * D], BF16, tag="x")
        nc.scalar.dma_start(xt0[:], x[:, 0:SZ[0] * D])
        xts.append(xt0)
        pks = sp.tile([P, PK_N], BF16)
        nc.scalar.dma_start(pks[:], pk)
        for i in range(1, NP):
            xt = xp.tile([PX, SZ[i] * D], BF16, tag="x")
            nc.scalar.dma_start(
                xt[:], x[:, OFF[i] * D:(OFF[i] + SZ[i]) * D]
            )
            xts.append(xt)

        umv = pks[:, PK_UM:PK_UM + P]
        mfv = pks[:, PK_MF:PK_MF + 2 * G].bitcast(F32)
        wmv = pks[:, PK_WM:PK_WM + 2 * G].bitcast(F32)
        amv = pks[:, PK_AM:PK_AM + 2 * G].bitcast(F32)

        # activation-table warm: no data deps, runs in the DMA shadow
        warm0 = sp.tile([P, 1], F32)
        nc.vector.memset(warm0[:], 1.0)
        warmo = sp.tile([P, 2], F32)
        nc.scalar.activation(warmo[:, 0:1], warm0[:], ACT.Exp)

        # replicate w MAXK times on-chip (bf16 copies run at 4x)
        wrep = sp.tile([P, MAXK * D], BF16)
        nc.vector.tensor_copy(wrep[:, 0:D], pws[:])
        rep = 1
        while rep < MAXK:
            n = min(rep, MAXK - rep)
            nc.vector.tensor_copy(
                wrep[:, rep * D:(rep + n) * D], wrep[:, 0:n * D]
            )
            rep += n
        w3 = wrep[:].rearrange("p (l d) -> p l d", d=D)

        # xw[p, t] = sum_d x[p, t, d] * w[d].  Only the PX occupied
        # partition rows are streamed/multiplied; the zero-padded rest
        # of xw is memset once so the exp sees finite values there.
        xw = sp.tile([P, G], F32)
        res = sp.tile([P, 2], F32)
        if PX < P:
            nc.vector.memset(xw[:], 0.0)
        for i in range(NP):
            k = SZ[i]
            x3 = xts[i][:].rearrange("p (l d) -> p l d", d=D)
            nc.vector.tensor_tensor(x3, x3, w3[0:PX, 0:k, :], mult)
            if k <= 2:
                nc.vector.tensor_reduce(
                    xw[0:PX, OFF[i]:OFF[i] + k], x3, axis=AX, op=add
                )
            else:
                h1 = hp.tile([PX, MAXK * 128], BF16, tag="h1")
                h13 = h1[:, 0:k * 128].rearrange("p (l d) -> p l d", d=128)
                h2 = hp.tile([PX, MAXK * 64], BF16, tag="h2")
                h23 = h2[:, 0:k * 64].rearrange("p (l d) -> p l d", d=64)
                nc.vector.tensor_tensor(h13, x3[:, :, 0:128], x3[:, :, 128:256], add)
                nc.vector.tensor_tensor(h23, h13[:, :, 0:64], h13[:, :, 64:128], add)
                nc.vector.tensor_reduce(
                    xw[0:PX, OFF[i]:OFF[i] + k], h23, axis=AX, op=add
                )
            if i == 2:
                nc.vector.tensor_reduce(res[:, 1:2], mfv, axis=AX, op=add)

        # fold the mask in additively; padded rows carry x=0 -> xw=0,
        # amask=-30 -> exp ~1e-13, so everything stays finite
        nc.vector.tensor_tensor(xw[:], xw[:], amv, add)

        em = sp.tile([P, G], F32)
        tot = sp.tile([P, 1], F32)
        nc.scalar.activation(em[:], xw[:], ACT.Exp, accum_out=tot[:])

        # cross-sub-chunk exclusive suffix of totals via one bf16 matmul
        tot_bf = sp.tile([P, 1], BF16)
        nc.vector.tensor_copy(tot_bf[:], tot[:])
        aps = pp.tile([P, 1], F32, tag="mm")
        nc.tensor.matmul(aps[:], umv, tot_bf[:], start=True, stop=True)

        # within-sub-chunk suffix sums, seeded from PSUM
        ss = sp.tile([P, G], F32)
        nc.vector.tensor_tensor_scan(
            ss[:][:, ::-1], em[:][:, ::-1], em[:][:, ::-1],
            initial=aps[:], op0=add, op1=bypass,
        )
        lt = sp.tile([P, G], F32)
        nc.scalar.activation(lt[:], ss[:], ACT.Ln)

        diff = sp.tile([P, G], F32)
        nc.vector.tensor_sub(diff[:], lt[:], xw[:])
        nc.vector.scalar_tensor_tensor(
            out=diff[:], in0=diff[:], scalar=1.0, in1=wmv,
            op0=bypass, op1=mult, accum_out=res[:, 0:1],
        )
        nc.sync.dma_start(out, res[:], single_packet=True)

    nc.compile()
    return nc


def make_in_maps(enc, mask, w_fc):
    import ml_dtypes

    bf = ml_dtypes.bfloat16
    lens = mask.sum(axis=1).astype(np.int64)          # [B]

    # pick the smallest sub-chunk size whose balanced packing fits the
    # 128 slots per core; batches stay whole on one core
    for G in sorted(_SZ):
        nch = -(-lens // G)                            # ceil(len/G) per batch
        order = np.argsort(-nch)
        core_of = np.empty(B, np.int64)
        load = np.zeros(NCORES, np.int64)
        ok = True
        for b in order:
            c = int(np.argmin(load))
            if load[c] + nch[b] > P:
                ok = False
                break
            core_of[b] = c
            load[c] += nch[b]
        if ok:
            break
    assert ok, "no sub-chunk size fits the 128 slots per core"
    # stream only occupied partition rows (multiple of 16 for the DMA
    # engine swizzle)
    PX = min(P, int(-(-int(load.max()) // 16) * 16))

    w_bits = np.tile(w_fc[HID:].astype(bf).view(np.uint16)[None, :], (P, 1))
    PK_UM = 0
    PK_MF = PK_UM + P
    PK_WM = PK_MF + 2 * G
    PK_AM = PK_WM + 2 * G
    PK_N = PK_AM + 2 * G

    in_maps = []
    for c in range(NCORES):
        slots = [
            (b, sc)
            for b in range(B) if core_of[b] == c
            for sc in range(int(nch[b]))
        ]
        xbuf = np.zeros((PX, G, D), np.float32)
        mf = np.zeros((P, G), np.float32)
        wm = np.zeros((P, G), np.float32)
        sb = np.full(P, -1, np.int64)
        ssc = np.zeros(P, np.int64)
        for s, (b, sc) in enumerate(slots):
            t0 = sc * G
            t1 = min(t0 + G, T)
            xbuf[s, 0:t1 - t0] = enc[b, t0:t1]
            tt = np.arange(t0, t0 + G)
            mf[s] = (tt < lens[b]).astype(np.float32)
            wm[s] = ((tt >= 1) & (tt < lens[b])).astype(np.float32)
            sb[s] = b
            ssc[s] = sc
        am = (mf - 1.0) * NEGM
        um = (
            (sb[:, None] == sb[None, :]) & (sb[:, None] >= 0)
            & (ssc[:, None] > ssc[None, :])
        ).astype(np.float32)
        pack = np.empty((P, PK_N), np.uint16)
        pack[:, PK_UM:PK_UM + P] = um.astype(bf).view(np.uint16)
        pack[:, PK_MF:PK_MF + 2 * G] = mf.view(np.uint16)
        pack[:, PK_WM:PK_WM + 2 * G] = wm.view(np.uint16)
        pack[:, PK_AM:PK_AM + 2 * G] = am.view(np.uint16)
        in_maps.append({
            "x": np.ascontiguousarray(xbuf.reshape(PX, G * D).astype(bf)),
            "pw": w_bits.view(bf),
            "pk": pack.view(bf),
        })
    return G, PX, in_maps


def kernel(**inputs) -> np.ndarray:
    enc = np.ascontiguousarray(np.asarray(inputs["encoder_output"], np.float32))
    mask = np.ascontiguousarray(np.asarray(inputs["mask"], np.int32))
    w_fc = np.asarray(inputs["w_fc"], np.float32)

    G, PX, in_maps = make_in_maps(enc, mask, w_fc)
    if (G, PX) not in _cache:
        _cache[(G, PX)] = _build_nc(G, PX)
    nc = _cache[(G, PX)]

    res = bass_utils.run_bass_kernel_spmd(
        nc, in_maps, core_ids=list(range(NCORES))
    )
    o = np.stack([r["out"] for r in res.results]).astype(np.float64)
    num = o[:, :, 0].sum()
    den = o[:, :, 1].sum() - B
    return np.asarray(num / den, dtype=np.float32)
